# revision 4
# baseline (speedup 1.0000x reference)
"""Trainium2 Bass kernel for BayesConcatSheafLearner edge message passing.

Computes, for each edge e=(u,v):
    maps_mean[e] = w_mean @ concat(x[u], x[v])
    maps_var[e]  = w_var  @ concat(x[u], x[v])

Strategy (8 NeuronCores, SPMD, edges sharded 100k/core):
  - Node rows are fetched with dma_gather (SWDGE custom descriptor-gen op).
    Indices are int16 used base-relative around row 25000, so the whole
    50k-node table is addressable in one pass; 4 SWDGE queues speed up Q7
    descriptor generation. x is gathered in fp16 (256B rows) to halve
    gather bytes — the Q7 gather op is the critical path.
  - Gathered [edge, chan] tiles are transposed 128x128 on the tensor
    engine (fp16, full rate) and contracted against the concatenated
    weight matrices: out[e, 0:128|128:256] = xr^T W1 + xc^T W2, fp16
    matmuls accumulating in fp32 PSUM.
  - PSUM results stage through SBUF (alternating Scalar/Vector engines)
    and stream back as [e, 256] f32 rows.
"""
import numpy as np

import concourse.bass as bass
import concourse.bacc as bacc
import concourse.mybir as mybir
from concourse import bass_utils
from concourse.tile import TileContext

N_NODES = 50000
C = 128
E_TOTAL = 800000
N_CORES = 8
E_PER_CORE = E_TOTAL // N_CORES          # 100000
N_REAL = 2048                            # real edges per gather op (16 subtiles)
N_PADS = 16                              # trailing pad idx slots (>=0 guard)
N_OP = N_REAL + N_PADS                   # 2064 descriptors per gather
T_SUB = N_REAL // 128                    # 16 compute subtiles
T_DST = (N_OP + 127) // 128              # 17 dst slots (last partly stale)
OPS = (E_PER_CORE + N_REAL - 1) // N_REAL  # 49
IDX_COLS = N_OP // 16                    # 129
BASE = 25000                             # gather base row (signed int16 reach)
PAD_IDX = 7                              # pad index (>=0, valid row)
OUT_ROWS = OPS * N_REAL                  # 100352

f32 = mybir.dt.float32
f16 = mybir.dt.float16
i16 = mybir.dt.int16

_prog_cache = {}


def _build_program():
    nc = bacc.Bacc(num_swdge_queues=4, dynamic_dma_scratch_size=49152)
    x = nc.declare_dram_parameter("x", [N_NODES, C], f16, isOutput=False)
    w1 = nc.declare_dram_parameter("w1", [C, 256], f16, isOutput=False)
    w2 = nc.declare_dram_parameter("w2", [C, 256], f16, isOutput=False)
    ir = nc.declare_dram_parameter("ir", [128, OPS * IDX_COLS], i16, isOutput=False)
    ic = nc.declare_dram_parameter("ic", [128, OPS * IDX_COLS], i16, isOutput=False)
    out = nc.declare_dram_parameter("out", [OUT_ROWS, 256], f32, isOutput=True)
    out_v = out[:].rearrange("(o t p) c -> o p t c", t=T_SUB, p=128)
    x_base = x[BASE:, :]

    with TileContext(nc) as tc:
        with (
            tc.tile_pool(name="const", bufs=1) as cpool,
            tc.tile_pool(name="gath", bufs=3) as gpool,
            tc.tile_pool(name="tr", bufs=6) as tpool,
            tc.tile_pool(name="ostage", bufs=3) as opool,
            tc.tile_pool(name="psum", bufs=4, space="PSUM") as ppool,
        ):
            ident = cpool.tile([128, 128], f16, tag="ident")
            nc.vector.memset(ident[:], 0.0)
            nc.gpsimd.affine_select(
                out=ident[:], in_=ident[:],
                compare_op=mybir.AluOpType.not_equal, fill=1.0,
                base=0, pattern=[[-1, 128]], channel_multiplier=1)
            w1_sb = cpool.tile([C, 256], f16, tag="w1")
            w2_sb = cpool.tile([C, 256], f16, tag="w2")
            nc.sync.dma_start(out=w1_sb[:], in_=w1[:])
            nc.sync.dma_start(out=w2_sb[:], in_=w2[:])

            for op in range(OPS):
                isl = slice(op * IDX_COLS, (op + 1) * IDX_COLS)
                ir_t = gpool.tile([128, IDX_COLS], i16, tag="irt")
                ic_t = gpool.tile([128, IDX_COLS], i16, tag="ict")
                nc.sync.dma_start(out=ir_t[:], in_=ir[:, isl])
                nc.sync.dma_start(out=ic_t[:], in_=ic[:, isl])
                dr = gpool.tile([128, T_DST * C], f16, tag="dr")
                dc = gpool.tile([128, T_DST * C], f16, tag="dc")
                nc.gpsimd.dma_gather(
                    dr[:].rearrange("p (t e) -> p t e", e=C), x_base,
                    ir_t[:], N_OP, N_OP, C,
                    single_packet=False, queue_num=(2 * op) % 4)
                nc.gpsimd.dma_gather(
                    dc[:].rearrange("p (t e) -> p t e", e=C), x_base,
                    ic_t[:], N_OP, N_OP, C,
                    single_packet=False, queue_num=(2 * op + 1) % 4)
                stage = opool.tile([128, T_SUB * 256], f32, tag="stage")
                for t in range(T_SUB):
                    ps_r = ppool.tile([128, C], f16, tag="pst")
                    ps_c = ppool.tile([128, C], f16, tag="pst")
                    nc.tensor.transpose(
                        out=ps_r[:], in_=dr[:, t * C:(t + 1) * C],
                        identity=ident[:])
                    nc.tensor.transpose(
                        out=ps_c[:], in_=dc[:, t * C:(t + 1) * C],
                        identity=ident[:])
                    xrT = tpool.tile([C, 128], f16, tag="xrT")
                    xcT = tpool.tile([C, 128], f16, tag="xcT")
                    # alternate copy engines to balance DVE/ACT load
                    eng_a = nc.vector if t % 2 == 0 else nc.scalar
                    eng_b = nc.scalar if t % 2 == 0 else nc.vector
                    if eng_a is nc.vector:
                        nc.vector.tensor_copy(out=xrT[:], in_=ps_r[:])
                    else:
                        nc.scalar.copy(out=xrT[:], in_=ps_r[:])
                    if eng_b is nc.vector:
                        nc.vector.tensor_copy(out=xcT[:], in_=ps_c[:])
                    else:
                        nc.scalar.copy(out=xcT[:], in_=ps_c[:])
                    ps_o = ppool.tile([128, 256], f32, tag="pso")
                    nc.tensor.matmul(out=ps_o[:], lhsT=xrT[:], rhs=w1_sb[:],
                                     start=True, stop=False)
                    nc.tensor.matmul(out=ps_o[:], lhsT=xcT[:], rhs=w2_sb[:],
                                     start=False, stop=True)
                    osl = stage[:, t * 256:(t + 1) * 256]
                    if t % 2 == 0:
                        nc.scalar.copy(out=osl, in_=ps_o[:])
                    else:
                        nc.vector.tensor_copy(out=osl, in_=ps_o[:])
                nc.sync.dma_start(out=out_v[op], in_=stage[:].rearrange(
                    "p (t c) -> p t c", c=256))
    nc.finalize()
    return nc


def _wrap_indices(idx_ops):
    """[OPS, N_OP] int16 -> [128, OPS*IDX_COLS] wrapped-in-16, replicated x8."""
    w = idx_ops.reshape(OPS, IDX_COLS, 16).transpose(2, 0, 1).reshape(
        16, OPS * IDX_COLS)
    return np.tile(w, (8, 1)).copy()


def _prep_core_indices(u):
    """u: [E_PER_CORE] node ids -> wrapped int16 index plane."""
    upad = np.full(OPS * N_REAL, BASE + PAD_IDX, dtype=np.int64)
    upad[:u.shape[0]] = u
    idx = (upad - BASE).astype(np.int16).reshape(OPS, N_REAL)
    pads = np.full((OPS, N_PADS), PAD_IDX, dtype=np.int16)
    return _wrap_indices(np.concatenate([idx, pads], axis=1))


def kernel(x, edge_index, w_mean, w_var):
    x = np.asarray(x, dtype=np.float32)
    edge_index = np.asarray(edge_index).astype(np.int64)
    w_mean = np.asarray(w_mean, dtype=np.float32)
    w_var = np.asarray(w_var, dtype=np.float32)

    x16 = np.ascontiguousarray(x.astype(np.float16))
    w1 = np.ascontiguousarray(
        np.concatenate([w_mean[:, :C].T, w_var[:, :C].T], axis=1)
    ).astype(np.float16)
    w2 = np.ascontiguousarray(
        np.concatenate([w_mean[:, C:].T, w_var[:, C:].T], axis=1)
    ).astype(np.float16)

    in_maps = []
    for k in range(N_CORES):
        sl = slice(k * E_PER_CORE, (k + 1) * E_PER_CORE)
        in_maps.append(dict(
            x=x16, w1=w1, w2=w2,
            ir=_prep_core_indices(edge_index[0, sl]),
            ic=_prep_core_indices(edge_index[1, sl]),
        ))

    if "nc" not in _prog_cache:
        _prog_cache["nc"] = _build_program()
    res = bass_utils.run_bass_kernel_spmd(
        _prog_cache["nc"], in_maps, core_ids=list(range(N_CORES)))

    maps_mean = np.empty((E_TOTAL, 128), dtype=np.float32)
    maps_var = np.empty((E_TOTAL, 128), dtype=np.float32)
    for k in range(N_CORES):
        valid = res.results[k]["out"][:E_PER_CORE]
        sl = slice(k * E_PER_CORE, (k + 1) * E_PER_CORE)
        maps_mean[sl] = valid[:, :128]
        maps_var[sl] = valid[:, 128:]
    return (maps_mean, maps_var)


# revision 6
# speedup vs baseline: 1.1311x; 1.1311x over previous
"""Trainium2 Bass kernel for BayesConcatSheafLearner edge message passing.

Computes, for each edge e=(u,v):
    maps_mean[e] = w_mean @ concat(x[u], x[v])
    maps_var[e]  = w_var  @ concat(x[u], x[v])

Strategy (8 NeuronCores, SPMD, edges sharded 100k/core):
  - Node rows are fetched with dma_gather (SWDGE custom descriptor-gen op).
    Indices are int16 used base-relative around row 25000, so the whole
    50k-node table is addressable in one pass; 4 SWDGE queues speed up Q7
    descriptor generation. x is gathered in fp16 (256B rows) to halve
    gather bytes — the Q7 gather op is the critical path.
  - Gathered [edge, chan] tiles are transposed 128x128 on the tensor
    engine (fp16, full rate) and contracted against the concatenated
    weight matrices: out[e, 0:128|128:256] = xr^T W1 + xc^T W2, fp16
    matmuls accumulating in fp32 PSUM.
  - PSUM results stage through SBUF (alternating Scalar/Vector engines)
    and stream back as [e, 256] f32 rows.
"""
import numpy as np

import concourse.bass as bass
import concourse.bacc as bacc
import concourse.mybir as mybir
from concourse import bass_utils
from concourse.tile import TileContext

N_NODES = 50000
C = 128
E_TOTAL = 800000
N_CORES = 8
E_PER_CORE = E_TOTAL // N_CORES          # 100000
N_REAL = 2048                            # real edges per gather op (16 subtiles)
N_PADS = 16                              # trailing pad idx slots (>=0 guard)
N_OP = N_REAL + N_PADS                   # 2064 descriptors per gather
T_SUB = N_REAL // 128                    # 16 compute subtiles
T_DST = (N_OP + 127) // 128              # 17 dst slots (last partly stale)
OPS = (E_PER_CORE + N_REAL - 1) // N_REAL  # 49
IDX_COLS = N_OP // 16                    # 129
BASE = 25000                             # gather base row (signed int16 reach)
PAD_IDX = 7                              # pad index (>=0, valid row)
OUT_ROWS = OPS * N_REAL                  # 100352

f32 = mybir.dt.float32
f16 = mybir.dt.float16
i16 = mybir.dt.int16

_prog_cache = {}


def _build_program():
    nc = bacc.Bacc(num_swdge_queues=4, dynamic_dma_scratch_size=49152)
    x = nc.declare_dram_parameter("x", [N_NODES, C], f16, isOutput=False)
    w1 = nc.declare_dram_parameter("w1", [C, 256], f16, isOutput=False)
    w2 = nc.declare_dram_parameter("w2", [C, 256], f16, isOutput=False)
    ir = nc.declare_dram_parameter("ir", [128, OPS * IDX_COLS], i16, isOutput=False)
    ic = nc.declare_dram_parameter("ic", [128, OPS * IDX_COLS], i16, isOutput=False)
    out = nc.declare_dram_parameter("out", [OUT_ROWS, 256], f32, isOutput=True)
    out_v = out[:].rearrange("(o t p) c -> o p t c", t=T_SUB, p=128)
    x_base = x[BASE:, :]

    with TileContext(nc) as tc:
        with (
            tc.tile_pool(name="const", bufs=1) as cpool,
            tc.tile_pool(name="gath", bufs=4) as gpool,
            tc.tile_pool(name="tr", bufs=6) as tpool,
            tc.tile_pool(name="ostage", bufs=3) as opool,
            tc.tile_pool(name="psum", bufs=4, space="PSUM") as ppool,
        ):
            ident = cpool.tile([128, 128], f16, tag="ident")
            nc.vector.memset(ident[:], 0.0)
            nc.gpsimd.affine_select(
                out=ident[:], in_=ident[:],
                compare_op=mybir.AluOpType.not_equal, fill=1.0,
                base=0, pattern=[[-1, 128]], channel_multiplier=1)
            w1_sb = cpool.tile([C, 256], f16, tag="w1")
            w2_sb = cpool.tile([C, 256], f16, tag="w2")
            nc.sync.dma_start(out=w1_sb[:], in_=w1[:])
            nc.sync.dma_start(out=w2_sb[:], in_=w2[:])
            OPS_HEAD = 4
            HC = OPS_HEAD * IDX_COLS
            ir_h = cpool.tile([128, HC], i16, tag="irh")
            ic_h = cpool.tile([128, HC], i16, tag="ich")
            ir_sb = cpool.tile([128, OPS * IDX_COLS - HC], i16, tag="ir")
            ic_sb = cpool.tile([128, OPS * IDX_COLS - HC], i16, tag="ic")
            nc.sync.dma_start(out=ir_h[:], in_=ir[:, :HC])
            nc.sync.dma_start(out=ic_h[:], in_=ic[:, :HC])
            nc.sync.dma_start(out=ir_sb[:], in_=ir[:, HC:])
            nc.sync.dma_start(out=ic_sb[:], in_=ic[:, HC:])

            for op in range(OPS):
                if op < OPS_HEAD:
                    isl = slice(op * IDX_COLS, (op + 1) * IDX_COLS)
                    ir_ap, ic_ap = ir_h[:, isl], ic_h[:, isl]
                else:
                    isl = slice(op * IDX_COLS - HC, (op + 1) * IDX_COLS - HC)
                    ir_ap, ic_ap = ir_sb[:, isl], ic_sb[:, isl]
                dr = gpool.tile([128, T_DST * C], f16, tag="dr")
                dc = gpool.tile([128, T_DST * C], f16, tag="dc")
                nc.gpsimd.dma_gather(
                    dr[:].rearrange("p (t e) -> p t e", e=C), x_base,
                    ir_ap, N_OP, N_OP, C,
                    single_packet=False, queue_num=(2 * op) % 4)
                nc.gpsimd.dma_gather(
                    dc[:].rearrange("p (t e) -> p t e", e=C), x_base,
                    ic_ap, N_OP, N_OP, C,
                    single_packet=False, queue_num=(2 * op + 1) % 4)
                stage = opool.tile([128, T_SUB * 256], f32, tag="stage")
                for t in range(T_SUB):
                    ps_r = ppool.tile([128, C], f16, tag="pst")
                    ps_c = ppool.tile([128, C], f16, tag="pst")
                    nc.tensor.transpose(
                        out=ps_r[:], in_=dr[:, t * C:(t + 1) * C],
                        identity=ident[:])
                    nc.tensor.transpose(
                        out=ps_c[:], in_=dc[:, t * C:(t + 1) * C],
                        identity=ident[:])
                    xrT = tpool.tile([C, 128], f16, tag="xrT")
                    xcT = tpool.tile([C, 128], f16, tag="xcT")
                    # alternate copy engines to balance DVE/ACT load
                    eng_a = nc.vector if t % 2 == 0 else nc.scalar
                    eng_b = nc.scalar if t % 2 == 0 else nc.vector
                    if eng_a is nc.vector:
                        nc.vector.tensor_copy(out=xrT[:], in_=ps_r[:])
                    else:
                        nc.scalar.copy(out=xrT[:], in_=ps_r[:])
                    if eng_b is nc.vector:
                        nc.vector.tensor_copy(out=xcT[:], in_=ps_c[:])
                    else:
                        nc.scalar.copy(out=xcT[:], in_=ps_c[:])
                    ps_o = ppool.tile([128, 256], f32, tag="pso")
                    nc.tensor.matmul(out=ps_o[:], lhsT=xrT[:], rhs=w1_sb[:],
                                     start=True, stop=False)
                    nc.tensor.matmul(out=ps_o[:], lhsT=xcT[:], rhs=w2_sb[:],
                                     start=False, stop=True)
                    osl = stage[:, t * 256:(t + 1) * 256]
                    if t % 2 == 0:
                        nc.scalar.copy(out=osl, in_=ps_o[:])
                    else:
                        nc.vector.tensor_copy(out=osl, in_=ps_o[:])
                nc.sync.dma_start(out=out_v[op], in_=stage[:].rearrange(
                    "p (t c) -> p t c", c=256))
    nc.finalize()
    return nc


def _wrap_indices(idx_ops):
    """[OPS, N_OP] int16 -> [128, OPS*IDX_COLS] wrapped-in-16, replicated x8."""
    w = idx_ops.reshape(OPS, IDX_COLS, 16).transpose(2, 0, 1).reshape(
        16, OPS * IDX_COLS)
    return np.tile(w, (8, 1)).copy()


def _prep_core_indices(u):
    """u: [E_PER_CORE] node ids -> wrapped int16 index plane."""
    upad = np.full(OPS * N_REAL, BASE + PAD_IDX, dtype=np.int64)
    upad[:u.shape[0]] = u
    idx = (upad - BASE).astype(np.int16).reshape(OPS, N_REAL)
    pads = np.full((OPS, N_PADS), PAD_IDX, dtype=np.int16)
    return _wrap_indices(np.concatenate([idx, pads], axis=1))


def kernel(x, edge_index, w_mean, w_var):
    x = np.asarray(x, dtype=np.float32)
    edge_index = np.asarray(edge_index).astype(np.int64)
    w_mean = np.asarray(w_mean, dtype=np.float32)
    w_var = np.asarray(w_var, dtype=np.float32)

    x16 = np.ascontiguousarray(x.astype(np.float16))
    w1 = np.ascontiguousarray(
        np.concatenate([w_mean[:, :C].T, w_var[:, :C].T], axis=1)
    ).astype(np.float16)
    w2 = np.ascontiguousarray(
        np.concatenate([w_mean[:, C:].T, w_var[:, C:].T], axis=1)
    ).astype(np.float16)

    in_maps = []
    for k in range(N_CORES):
        sl = slice(k * E_PER_CORE, (k + 1) * E_PER_CORE)
        in_maps.append(dict(
            x=x16, w1=w1, w2=w2,
            ir=_prep_core_indices(edge_index[0, sl]),
            ic=_prep_core_indices(edge_index[1, sl]),
        ))

    if "nc" not in _prog_cache:
        _prog_cache["nc"] = _build_program()
    res = bass_utils.run_bass_kernel_spmd(
        _prog_cache["nc"], in_maps, core_ids=list(range(N_CORES)))

    maps_mean = np.empty((E_TOTAL, 128), dtype=np.float32)
    maps_var = np.empty((E_TOTAL, 128), dtype=np.float32)
    for k in range(N_CORES):
        valid = res.results[k]["out"][:E_PER_CORE]
        sl = slice(k * E_PER_CORE, (k + 1) * E_PER_CORE)
        maps_mean[sl] = valid[:, :128]
        maps_var[sl] = valid[:, 128:]
    return (maps_mean, maps_var)
